# revision 3
# baseline (speedup 1.0000x reference)
"""Trainium2 8-core attention kernel (nn_Attention_19954418057485).

Sharding: heads are split across the 8 cores (2 heads = 128 channels each);
every core processes both batch elements for its heads.  After attention,
an AllToAll over all 8 cores swaps the channel axis for the row axis, so
each core finishes the full output projection for its 512 rows of the
flattened (B*N, C) output.

Per-core pipeline (all matmuls on PE, exp on ACT, elementwise on DVE):
  x^T (bf16)  --PE-->  q,k (rows,ch) + v^T        [QKV projection]
  q,k: LayerNorm (d=64) + RoPE on DVE, rounded to fp32r
  PE transpose -> q^T, k^T [ch, n];  v^T -> V [n, ch] with a ones column
  per (batch, head): S^T = K Q^T (fp32r), exp on ACT (scale=1/8, fp32r),
  AV accumulates V_ext^T @ expS^T giving out^T rows 0..63 and the softmax
  denominator in row 64 (from the ones column); normalize via reciprocal +
  PE outer-product broadcast.
  AllToAll (bf16) -> full-channel rows -> output projection (bf16) + bias.
"""
import sys

if "/opt/trn_rl_repo" not in sys.path:
    sys.path.insert(0, "/opt/trn_rl_repo")

import numpy as np
import ml_dtypes

import concourse.bass as bass
import concourse.tile as tile
from concourse import mybir
from concourse.bass_utils import run_bass_kernel_spmd

N_CORES = 8
B, N, C, H = 2, 2048, 1024, 16
D = 64
HPC = H // N_CORES          # heads per core = 2
CPC = HPC * D               # channels per core = 128
NTOT = B * N                # 4096 flattened rows
RPC = NTOT // N_CORES       # output rows per core = 512
EPS = 1e-6

BF16 = mybir.dt.bfloat16
F32 = mybir.dt.float32
F32R = mybir.dt.float32r
AF = mybir.ActivationFunctionType
OP = mybir.AluOpType
AX = mybir.AxisListType


def _split_excess_waits(nc, max_waits=1):
    """walrus rejects instructions with more than a couple of sem-wait
    commands; split extras onto preceding same-engine NoOps."""
    for fn in nc.m.functions:
        for blk in fn.blocks:
            new_insts = []
            for ins in blk.instructions:
                si = ins.sync_info
                ow = list(si.on_wait) if si is not None and si.on_wait else []
                if len(ow) > max_waits:
                    head = ow[: len(ow) - max_waits]
                    rest = ow[len(ow) - max_waits:]
                    for i in range(0, len(head), max_waits):
                        new_insts.append(mybir.InstNoOp(
                            name=f"{ins.name}_ws{i}",
                            engine=ins.engine,
                            ins=[], outs=[],
                            sync_info=mybir.SyncInfo(
                                on_wait=head[i:i + max_waits], on_update=[]),
                        ))
                    ins.sync_info = mybir.SyncInfo(
                        on_wait=rest, on_update=list(si.on_update or []))
                new_insts.append(ins)
            blk.instructions = new_insts


def build():
    nc = bass.Bass("TRN2", target_bir_lowering=False, debug=False,
                   num_devices=N_CORES)
    xT_d = nc.dram_tensor("xT", (C, NTOT), BF16, kind="ExternalInput")
    wqkT_d = nc.dram_tensor("wqkT", (C, 2 * CPC), BF16, kind="ExternalInput")
    wvT_d = nc.dram_tensor("wvT", (C, CPC), BF16, kind="ExternalInput")
    wpT_d = nc.dram_tensor("wpT", (C, C), BF16, kind="ExternalInput")
    bias_d = nc.dram_tensor("biasb", (128, C), F32, kind="ExternalInput")
    cos_d = nc.dram_tensor("cosd", (N, D), F32, kind="ExternalInput")
    sin_d = nc.dram_tensor("sind", (N, D), F32, kind="ExternalInput")
    ident_d = nc.dram_tensor("identd", (128, 128), F32, kind="ExternalInput")
    out_d = nc.dram_tensor("out", (RPC, C), F32, kind="ExternalOutput")

    with tile.TileContext(nc) as tc:
        with tc.tile_pool(name="consts", bufs=1) as consts, \
             tc.tile_pool(name="xload", bufs=2) as xload, \
             tc.tile_pool(name="freqs", bufs=2) as freqs, \
             tc.tile_pool(name="work", bufs=2) as work, \
             tc.tile_pool(name="small", bufs=3) as small, \
             tc.tile_pool(name="exps", bufs=4) as expp, \
             tc.tile_pool(name="norm", bufs=3) as normp, \
             tc.tile_pool(name="ps", bufs=5, space="PSUM") as ps, \
             tc.tile_pool(name="psav", bufs=2, space="PSUM") as psav, \
             tc.tile_pool(name="dram", bufs=1, space="DRAM") as dram:

            # ---- constants -------------------------------------------------
            wqk_sb = consts.tile([128, 8, 2 * CPC], BF16)
            nc.sync.dma_start(wqk_sb[:],
                              wqkT_d.ap().rearrange("(co p) k -> p co k", p=128))
            wv_sb = consts.tile([128, 8, CPC], BF16)
            nc.sync.dma_start(wv_sb[:],
                              wvT_d.ap().rearrange("(co p) k -> p co k", p=128))
            wp_sb = consts.tile([128, 8, C], BF16)
            nc.sync.dma_start(wp_sb[:],
                              wpT_d.ap().rearrange("(co p) k -> p co k", p=128))
            bias_sb = consts.tile([128, C], F32)
            nc.sync.dma_start(bias_sb[:], bias_d.ap())
            cos_r = cos_d.ap().rearrange("(c p) d -> p c d", p=128)
            sin_r = sin_d.ap().rearrange("(c p) d -> p c d", p=128)
            ident_f = consts.tile([128, 128], F32)
            nc.sync.dma_start(ident_f[:], ident_d.ap())
            identr = consts.tile([128, 128], F32R)
            nc.scalar.activation(identr[:], ident_f[:], AF.Copy)
            onesr = consts.tile([1, 64], F32R)
            nc.scalar.activation(onesr[:], ident_f[0:1, 0:64], AF.Identity,
                                 scale=0.0, bias=1.0)

            # ---- persistent tensors ---------------------------------------
            qT = consts.tile([128, NTOT], F32R)       # [ch(2 heads), b*N+n]
            kT = consts.tile([128, NTOT], F32R)
            # V with ones column: [n%128, chunk, head, 64 d + 1 one + pad]
            vext = consts.tile([128, 32, HPC, 66], F32R)
            nc.scalar.activation(
                vext[:, :, :, 64:65],
                bias_sb[:, 0:64].rearrange("p (a b c) -> p a b c", a=32, b=2),
                AF.Identity, scale=0.0, bias=1.0)
            attn_outT = consts.tile([128, NTOT], BF16)

            xT_r = xT_d.ap().rearrange("(co p) n -> p co n", p=128)

            def preamble_tile(t):
                """QKV proj + LN + RoPE + transposes for n-rows
                [t*512, (t+1)*512)."""
                xt = xload.tile([128, 8, 512], BF16, tag="xt", name=f"xt{t}")
                nc.sync.dma_start(xt[:], xT_r[:, :, 512 * t:512 * (t + 1)])

                qk_nd = work.tile([128, 4, 4, D], F32, tag="qknd",
                                  name=f"qknd{t}")
                for ns in range(4):
                    pj = ps.tile([128, 2 * CPC], F32, tag="ps",
                                 name=f"pj{t}_{ns}")
                    for cc in range(8):
                        nc.tensor.matmul(pj[:],
                                         xt[:, cc, 128 * ns:128 * (ns + 1)],
                                         wqk_sb[:, cc, :],
                                         start=(cc == 0), stop=(cc == 7))
                    nc.vector.tensor_copy(
                        qk_nd[:, ns],
                        pj[:].rearrange("p (s d) -> p s d", s=4))

                pv = ps.tile([128, 512], F32, tag="ps", name=f"pv{t}")
                for cc in range(8):
                    nc.tensor.matmul(pv[:], wv_sb[:, cc, :], xt[:, cc, :],
                                     start=(cc == 0), stop=(cc == 7))
                vts = xload.tile([128, 512], F32R, tag="vts", name=f"vts{t}")
                nc.vector.tensor_copy(vts[:], pv[:])

                # LayerNorm stats over d=64 for each (row, slot)
                s1 = small.tile([128, 4, 4], F32, tag="s1", name=f"s1_{t}")
                nc.vector.reduce_sum(s1[:], qk_nd[:], axis=AX.X)
                sq = work.tile([128, 4, 4, D], F32, tag="tmp", name=f"sq{t}")
                nc.vector.tensor_tensor(sq[:], qk_nd[:], qk_nd[:], OP.mult)
                s2 = small.tile([128, 4, 4], F32, tag="s2", name=f"s2_{t}")
                nc.vector.reduce_sum(s2[:], sq[:], axis=AX.X)
                mu = small.tile([128, 4, 4], F32, tag="mu", name=f"mu{t}")
                nc.vector.tensor_scalar_mul(mu[:], s1[:], 1.0 / D)
                var = small.tile([128, 4, 4], F32, tag="var", name=f"var{t}")
                nc.vector.tensor_scalar_mul(var[:], s2[:], 1.0 / D)
                mm = small.tile([128, 4, 4], F32, tag="mm", name=f"mm{t}")
                nc.vector.tensor_tensor(mm[:], mu[:], mu[:], OP.mult)
                nc.vector.tensor_tensor(var[:], var[:], mm[:], OP.subtract)
                nc.vector.tensor_scalar_add(var[:], var[:], EPS)
                # rsqrt(var+eps) = exp(-0.5*ln(var+eps)): stays in the
                # ln/exp ACT table set (no thrash against attention's Exp)
                lnv = small.tile([128, 4, 4], F32, tag="lnv", name=f"lnv{t}")
                nc.scalar.activation(lnv[:], var[:], AF.Ln)
                a_ = small.tile([128, 4, 4], F32, tag="a", name=f"a{t}")
                nc.scalar.activation(a_[:], lnv[:], AF.Exp, scale=-0.5)
                nma = small.tile([128, 4, 4], F32, tag="nma", name=f"nma{t}")
                nc.vector.tensor_tensor(nma[:], mu[:], a_[:], OP.mult)
                # qn = q*a - mu*a
                nc.vector.tensor_tensor(
                    qk_nd[:], qk_nd[:],
                    a_[:, :, :, None].to_broadcast((128, 4, 4, D)), OP.mult)
                nc.vector.tensor_tensor(
                    qk_nd[:], qk_nd[:],
                    nma[:, :, :, None].to_broadcast((128, 4, 4, D)),
                    OP.subtract)

                # RoPE: out = qn*cos + rot_half(qn)*sin  (fp32r output)
                tb = t % 4
                cos_t = freqs.tile([128, 4, D], F32, tag="cos", name=f"cos{t}")
                nc.sync.dma_start(cos_t[:], cos_r[:, 4 * tb:4 * (tb + 1), :])
                sin_t = freqs.tile([128, 4, D], F32, tag="sin", name=f"sin{t}")
                nc.sync.dma_start(sin_t[:], sin_r[:, 4 * tb:4 * (tb + 1), :])
                cs = cos_t[:, :, None, :].to_broadcast((128, 4, 4, D))
                sn0 = sin_t[:, :, None, 0:32].to_broadcast((128, 4, 4, 32))
                sn1 = sin_t[:, :, None, 32:64].to_broadcast((128, 4, 4, 32))
                tmp = work.tile([128, 4, 4, D], F32, tag="tmp", name=f"tmp{t}")
                nc.vector.tensor_tensor(tmp[:], qk_nd[:], cs, OP.mult)
                qk_r = work.tile([128, 4, 4, D], F32R, tag="qkr",
                                 name=f"qkr{t}")
                nc.vector.tensor_tensor(qk_r[:, :, :, 0:32],
                                        qk_nd[:, :, :, 32:64], sn0, OP.mult)
                nc.vector.tensor_tensor(qk_r[:, :, :, 0:32],
                                        tmp[:, :, :, 0:32],
                                        qk_r[:, :, :, 0:32], OP.subtract)
                nc.vector.tensor_tensor(qk_r[:, :, :, 32:64],
                                        qk_nd[:, :, :, 0:32], sn1, OP.mult)
                nc.vector.tensor_tensor(qk_r[:, :, :, 32:64],
                                        tmp[:, :, :, 32:64],
                                        qk_r[:, :, :, 32:64], OP.add)

                # transposes: q,k -> [ch, n]; v^T -> V rows
                for j in range(4):
                    g = 4 * t + j
                    col = 128 * g
                    ptq = ps.tile([128, 128], F32R, tag="ps",
                                  name=f"ptq{g}")
                    nc.tensor.transpose(ptq[:], qk_r[:, j, 0:2, :], identr[:])
                    nc.vector.tensor_copy(qT[:, col:col + 128], ptq[:])
                    ptk = ps.tile([128, 128], F32R, tag="ps",
                                  name=f"ptk{g}")
                    nc.tensor.transpose(ptk[:], qk_r[:, j, 2:4, :], identr[:])
                    nc.vector.tensor_copy(kT[:, col:col + 128], ptk[:])
                    ptv = ps.tile([128, 128], F32R, tag="ps",
                                  name=f"ptv{g}")
                    nc.tensor.transpose(ptv[:], vts[:, 128 * j:128 * (j + 1)],
                                        identr[:])
                    nc.vector.tensor_copy(
                        vext[:, g, :, 0:64],
                        ptv[:].rearrange("p (h d) -> p h d", h=HPC))

            def attn_pair(b, h):
                """softmax(K Q^T / 8)^T-weighted V for (batch b, local head
                h); writes normalized out^T into attn_outT."""
                col0 = N * b
                hof = D * h
                for qt in range(4):
                    qs = col0 + 512 * qt
                    pav = psav.tile([65, 512], F32, tag="av",
                                    name=f"pav{b}{h}{qt}")
                    for kc in range(16):
                        pS = ps.tile([128, 512], F32, tag="ps",
                                     name=f"pS{b}{h}{qt}_{kc}")
                        nc.tensor.matmul(
                            pS[:],
                            kT[hof:hof + D,
                               col0 + 128 * kc:col0 + 128 * (kc + 1)],
                            qT[hof:hof + D, qs:qs + 512],
                            start=True, stop=True)
                        es = expp.tile([128, 512], F32R, tag="es",
                                       name=f"es{b}{h}{qt}_{kc}")
                        nc.scalar.activation(es[:], pS[:], AF.Exp, scale=0.125)
                        nc.tensor.matmul(pav[:],
                                         vext[:, 16 * b + kc, h, 0:65],
                                         es[:],
                                         start=(kc == 0), stop=(kc == 15))
                    # normalize columns by row 64 (softmax denominator)
                    rec = normp.tile([1, 512], F32, tag="rec",
                                     name=f"rec{b}{h}{qt}")
                    nc.vector.reciprocal(rec[:], pav[64:65, :])
                    recr = normp.tile([1, 512], F32R, tag="recr",
                                      name=f"recr{b}{h}{qt}")
                    nc.scalar.activation(recr[:], rec[:], AF.Copy)
                    pbc = ps.tile([64, 512], F32, tag="ps",
                                  name=f"pbc{b}{h}{qt}")
                    nc.tensor.matmul(pbc[:], onesr[:], recr[:],
                                     start=True, stop=True)
                    bc = normp.tile([64, 512], F32, tag="bc",
                                    name=f"bc{b}{h}{qt}")
                    nc.vector.tensor_copy(bc[:], pbc[:])
                    nc.vector.tensor_tensor(
                        attn_outT[hof:hof + D, qs:qs + 512],
                        pav[0:64, :], bc[:], OP.mult)

            # ---- emit: preamble pipelined with attention ------------------
            for t in range(4):
                preamble_tile(t)
            for h in range(HPC):
                attn_pair(0, h)
            for t in range(4, 8):
                preamble_tile(t)
            for h in range(HPC):
                attn_pair(1, h)

            # ---- AllToAll: swap channel axis for row axis -----------------
            ccin = dram.tile([N_CORES, 128, RPC], BF16)
            ccout = dram.tile([N_CORES, 128, RPC], BF16)
            for j in range(N_CORES):
                nc.sync.dma_start(ccin[j], attn_outT[:, RPC * j:RPC * (j + 1)])
            nc.gpsimd.collective_compute(
                "AllToAll", OP.bypass,
                replica_groups=[list(range(N_CORES))],
                ins=[ccin[:].opt()], outs=[ccout[:].opt()])
            ccout_r = ccout[:].rearrange("j p n -> p j n")
            out_r = out_d.ap().rearrange("(t p) o -> p t o", p=128)

            # ---- output projection for this core's 512 rows ---------------
            for nt in range(4):
                gat = xload.tile([128, 8, 128], BF16, tag="gat",
                                 name=f"gat{nt}")
                nc.sync.dma_start(gat[:],
                                  ccout_r[:, :, 128 * nt:128 * (nt + 1)])
                ob = work.tile([128, C], F32, tag="ob", name=f"ob{nt}")
                for hf in range(2):
                    po = ps.tile([128, 512], F32, tag="ps",
                                 name=f"po{nt}_{hf}")
                    for cc in range(8):
                        nc.tensor.matmul(
                            po[:],
                            gat[:, cc, :],
                            wp_sb[:, cc, 512 * hf:512 * (hf + 1)],
                            start=(cc == 0), stop=(cc == 7))
                    nc.vector.tensor_tensor(
                        ob[:, 512 * hf:512 * (hf + 1)], po[:],
                        bias_sb[:, 512 * hf:512 * (hf + 1)], OP.add)
                nc.sync.dma_start(out_r[:, nt, :], ob[:])
    _split_excess_waits(nc)
    return nc


_NC_CACHE = {}


def _get_nc():
    if "nc" not in _NC_CACHE:
        _NC_CACHE["nc"] = build()
    return _NC_CACHE["nc"]


def _prep_inputs(x, w_qkv, w_proj, b_proj, freqs_cos, freqs_sin):
    x = np.asarray(x, dtype=np.float32)
    w_qkv = np.asarray(w_qkv, dtype=np.float32)
    w_proj = np.asarray(w_proj, dtype=np.float32)
    b_proj = np.asarray(b_proj, dtype=np.float32)
    cos = np.asarray(freqs_cos, dtype=np.float32).reshape(N, D)
    sin = np.asarray(freqs_sin, dtype=np.float32).reshape(N, D)

    bf = ml_dtypes.bfloat16
    xT = np.ascontiguousarray(x.reshape(NTOT, C).T).astype(bf)
    wpT = np.ascontiguousarray(w_proj.T).astype(bf)
    biasb = np.ascontiguousarray(np.broadcast_to(b_proj, (128, C))).astype(np.float32)
    ident = np.eye(128, dtype=np.float32)

    in_maps = []
    for i in range(N_CORES):
        r0 = CPC * i
        wqk = np.concatenate([w_qkv[r0:r0 + CPC],
                              w_qkv[C + r0:C + r0 + CPC]], axis=0)
        wqkT = np.ascontiguousarray(wqk.T).astype(bf)
        wvT = np.ascontiguousarray(w_qkv[2 * C + r0:2 * C + r0 + CPC].T).astype(bf)
        in_maps.append({
            "xT": xT, "wqkT": wqkT, "wvT": wvT, "wpT": wpT,
            "biasb": biasb, "cosd": cos, "sind": sin, "identd": ident,
        })
    return in_maps


def kernel(x, w_qkv, w_proj, b_proj, freqs_cos, freqs_sin):
    in_maps = _prep_inputs(x, w_qkv, w_proj, b_proj, freqs_cos, freqs_sin)
    nc = _get_nc()
    res = run_bass_kernel_spmd(nc, in_maps, core_ids=list(range(N_CORES)))
    full = np.concatenate([res.results[i]["out"] for i in range(N_CORES)],
                          axis=0)
    return full.reshape(B, N, C).astype(np.float32)


# revision 5
# speedup vs baseline: 1.0875x; 1.0875x over previous
"""Trainium2 8-core attention kernel (nn_Attention_19954418057485).

Sharding: heads are split across the 8 cores (2 heads = 128 channels each);
every core processes both batch elements for its heads.  After attention,
an AllToAll over all 8 cores swaps the channel axis for the row axis, so
each core finishes the full output projection for its 512 rows of the
flattened (B*N, C) output.

Per-core pipeline (all matmuls on PE, exp on ACT, elementwise on DVE):
  x^T (bf16)  --PE-->  q,k (rows,ch) + v^T        [QKV projection]
  q,k: LayerNorm (d=64) + RoPE on DVE, rounded to fp32r
  PE transpose -> q^T, k^T [ch, n];  v^T -> V [n, ch] with a ones column
  per (batch, head): S^T = K Q^T (fp32r), exp on ACT (scale=1/8, fp32r),
  AV accumulates V_ext^T @ expS^T giving out^T rows 0..63 and the softmax
  denominator in row 64 (from the ones column); normalize via reciprocal +
  PE outer-product broadcast.
  AllToAll (bf16) -> full-channel rows -> output projection (bf16) + bias.
"""
import sys

if "/opt/trn_rl_repo" not in sys.path:
    sys.path.insert(0, "/opt/trn_rl_repo")

import numpy as np
import ml_dtypes

import concourse.bass as bass
import concourse.tile as tile
from concourse import mybir
from concourse.bass_utils import run_bass_kernel_spmd

N_CORES = 8
B, N, C, H = 2, 2048, 1024, 16
D = 64
HPC = H // N_CORES          # heads per core = 2
CPC = HPC * D               # channels per core = 128
NTOT = B * N                # 4096 flattened rows
RPC = NTOT // N_CORES       # output rows per core = 512
EPS = 1e-6

BF16 = mybir.dt.bfloat16
F32 = mybir.dt.float32
F32R = mybir.dt.float32r
AF = mybir.ActivationFunctionType
OP = mybir.AluOpType
AX = mybir.AxisListType


def _split_excess_waits(nc, max_waits=1):
    """walrus rejects instructions with more than a couple of sem-wait
    commands; split extras onto preceding same-engine NoOps."""
    for fn in nc.m.functions:
        for blk in fn.blocks:
            new_insts = []
            for ins in blk.instructions:
                si = ins.sync_info
                ow = list(si.on_wait) if si is not None and si.on_wait else []
                if len(ow) > max_waits:
                    head = ow[: len(ow) - max_waits]
                    rest = ow[len(ow) - max_waits:]
                    for i in range(0, len(head), max_waits):
                        new_insts.append(mybir.InstNoOp(
                            name=f"{ins.name}_ws{i}",
                            engine=ins.engine,
                            ins=[], outs=[],
                            sync_info=mybir.SyncInfo(
                                on_wait=head[i:i + max_waits], on_update=[]),
                        ))
                    ins.sync_info = mybir.SyncInfo(
                        on_wait=rest, on_update=list(si.on_update or []))
                new_insts.append(ins)
            blk.instructions = new_insts


def build():
    nc = bass.Bass("TRN2", target_bir_lowering=False, debug=False,
                   num_devices=N_CORES)
    xT_d = nc.dram_tensor("xT", (C, NTOT), BF16, kind="ExternalInput")
    wqkT_d = nc.dram_tensor("wqkT", (C, 2 * CPC), BF16, kind="ExternalInput")
    wvT_d = nc.dram_tensor("wvT", (C, CPC), BF16, kind="ExternalInput")
    wpT_d = nc.dram_tensor("wpT", (C, C), BF16, kind="ExternalInput")
    bias_d = nc.dram_tensor("biasb", (128, C), F32, kind="ExternalInput")
    cos_d = nc.dram_tensor("cosd", (N, D), F32, kind="ExternalInput")
    sin_d = nc.dram_tensor("sind", (N, D), F32, kind="ExternalInput")
    ident_d = nc.dram_tensor("identd", (128, 128), F32, kind="ExternalInput")
    out_d = nc.dram_tensor("out", (RPC, C), F32, kind="ExternalOutput")

    with tile.TileContext(nc) as tc:
        with tc.tile_pool(name="consts", bufs=1) as consts, \
             tc.tile_pool(name="xload", bufs=3) as xload, \
             tc.tile_pool(name="vpool", bufs=4) as vpool, \
             tc.tile_pool(name="qkrp", bufs=4) as qkrp, \
             tc.tile_pool(name="freqs", bufs=2) as freqs, \
             tc.tile_pool(name="work", bufs=2) as work, \
             tc.tile_pool(name="small", bufs=3) as small, \
             tc.tile_pool(name="exps", bufs=4) as expp, \
             tc.tile_pool(name="norm", bufs=2) as normp, \
             tc.tile_pool(name="ps", bufs=5, space="PSUM") as ps, \
             tc.tile_pool(name="psav", bufs=2, space="PSUM") as psav, \
             tc.tile_pool(name="dram", bufs=1, space="DRAM") as dram:

            # ---- constants -------------------------------------------------
            wqk_sb = consts.tile([128, 8, 2 * CPC], BF16)
            nc.sync.dma_start(wqk_sb[:],
                              wqkT_d.ap().rearrange("(co p) k -> p co k", p=128))
            wv_sb = consts.tile([128, 8, CPC], BF16)
            nc.sync.dma_start(wv_sb[:],
                              wvT_d.ap().rearrange("(co p) k -> p co k", p=128))
            wp_sb = consts.tile([128, 8, C], BF16)
            nc.sync.dma_start(wp_sb[:],
                              wpT_d.ap().rearrange("(co p) k -> p co k", p=128))
            bias_sb = consts.tile([128, C], F32)
            nc.sync.dma_start(bias_sb[:], bias_d.ap())
            cos_r = cos_d.ap().rearrange("(c p) d -> p c d", p=128)
            sin_r = sin_d.ap().rearrange("(c p) d -> p c d", p=128)
            ident_f = consts.tile([128, 128], F32)
            nc.sync.dma_start(ident_f[:], ident_d.ap())
            identr = consts.tile([128, 128], F32R)
            nc.scalar.activation(identr[:], ident_f[:], AF.Copy)
            onesr = consts.tile([1, 64], F32R)
            nc.scalar.activation(onesr[:], ident_f[0:1, 0:64], AF.Identity,
                                 scale=0.0, bias=1.0)

            # ---- persistent tensors ---------------------------------------
            qT = consts.tile([128, NTOT], F32R)       # [ch(2 heads), b*N+n]
            kT = consts.tile([128, NTOT], F32R)
            # V with ones column: [n%128, chunk, head, 64 d + 1 one + pad]
            vext = consts.tile([128, 32, HPC, 66], F32R)
            nc.scalar.activation(
                vext[:, :, :, 64:65],
                bias_sb[:, 0:64].rearrange("p (a b c) -> p a b c", a=32, b=2),
                AF.Identity, scale=0.0, bias=1.0)
            attn_outT = consts.tile([128, NTOT], BF16)

            xT_r = xT_d.ap().rearrange("(co p) n -> p co n", p=128)

            def preamble_tile(t):
                """QKV proj + LN + RoPE + transposes for n-rows
                [t*512, (t+1)*512)."""
                xt = xload.tile([128, 8, 512], BF16, tag="xt", name=f"xt{t}")
                nc.sync.dma_start(xt[:], xT_r[:, :, 512 * t:512 * (t + 1)])

                qk_nd = work.tile([128, 4, 4, D], F32, tag="qknd",
                                  name=f"qknd{t}")
                for ns in range(4):
                    pj = ps.tile([128, 2 * CPC], F32, tag="ps",
                                 name=f"pj{t}_{ns}")
                    for cc in range(8):
                        nc.tensor.matmul(pj[:],
                                         xt[:, cc, 128 * ns:128 * (ns + 1)],
                                         wqk_sb[:, cc, :],
                                         start=(cc == 0), stop=(cc == 7))
                    nc.vector.tensor_copy(
                        qk_nd[:, ns],
                        pj[:].rearrange("p (s d) -> p s d", s=4))

                pv = ps.tile([128, 512], F32, tag="ps", name=f"pv{t}")
                for cc in range(8):
                    nc.tensor.matmul(pv[:], wv_sb[:, cc, :], xt[:, cc, :],
                                     start=(cc == 0), stop=(cc == 7))
                vts = vpool.tile([128, 512], F32R, tag="vts", name=f"vts{t}")
                nc.vector.tensor_copy(vts[:], pv[:])

                # LayerNorm stats over d=64 for each (row, slot)
                s1 = small.tile([128, 4, 4], F32, tag="s1", name=f"s1_{t}")
                nc.vector.reduce_sum(s1[:], qk_nd[:], axis=AX.X)
                sq = work.tile([128, 4, 4, D], F32, tag="tmp", name=f"sq{t}")
                nc.vector.tensor_tensor(sq[:], qk_nd[:], qk_nd[:], OP.mult)
                s2 = small.tile([128, 4, 4], F32, tag="s2", name=f"s2_{t}")
                nc.vector.reduce_sum(s2[:], sq[:], axis=AX.X)
                mu = small.tile([128, 4, 4], F32, tag="mu", name=f"mu{t}")
                nc.vector.tensor_scalar_mul(mu[:], s1[:], 1.0 / D)
                var = small.tile([128, 4, 4], F32, tag="var", name=f"var{t}")
                nc.vector.tensor_scalar_mul(var[:], s2[:], 1.0 / D)
                mm = small.tile([128, 4, 4], F32, tag="mm", name=f"mm{t}")
                nc.vector.tensor_tensor(mm[:], mu[:], mu[:], OP.mult)
                nc.vector.tensor_tensor(var[:], var[:], mm[:], OP.subtract)
                nc.vector.tensor_scalar_add(var[:], var[:], EPS)
                # rsqrt(var+eps) = exp(-0.5*ln(var+eps)): stays in the
                # ln/exp ACT table set (no thrash against attention's Exp)
                lnv = small.tile([128, 4, 4], F32, tag="lnv", name=f"lnv{t}")
                nc.scalar.activation(lnv[:], var[:], AF.Ln)
                a_ = small.tile([128, 4, 4], F32, tag="a", name=f"a{t}")
                nc.scalar.activation(a_[:], lnv[:], AF.Exp, scale=-0.5)
                nma = small.tile([128, 4, 4], F32, tag="nma", name=f"nma{t}")
                nc.vector.tensor_tensor(nma[:], mu[:], a_[:], OP.mult)
                # qn = q*a - mu*a
                nc.vector.tensor_tensor(
                    qk_nd[:], qk_nd[:],
                    a_[:, :, :, None].to_broadcast((128, 4, 4, D)), OP.mult)
                nc.vector.tensor_tensor(
                    qk_nd[:], qk_nd[:],
                    nma[:, :, :, None].to_broadcast((128, 4, 4, D)),
                    OP.subtract)

                # RoPE: out = qn*cos + rot_half(qn)*sin  (fp32r output)
                tb = t % 4
                cos_t = freqs.tile([128, 4, D], F32, tag="cos", name=f"cos{t}")
                nc.sync.dma_start(cos_t[:], cos_r[:, 4 * tb:4 * (tb + 1), :])
                sin_t = freqs.tile([128, 4, D], F32, tag="sin", name=f"sin{t}")
                nc.sync.dma_start(sin_t[:], sin_r[:, 4 * tb:4 * (tb + 1), :])
                cs = cos_t[:, :, None, :].to_broadcast((128, 4, 4, D))
                sn0 = sin_t[:, :, None, 0:32].to_broadcast((128, 4, 4, 32))
                sn1 = sin_t[:, :, None, 32:64].to_broadcast((128, 4, 4, 32))
                tmp = work.tile([128, 4, 4, D], F32, tag="tmp", name=f"tmp{t}")
                nc.vector.tensor_tensor(tmp[:], qk_nd[:], cs, OP.mult)
                qk_r = qkrp.tile([128, 4, 4, D], F32R, tag="qkr",
                                 name=f"qkr{t}")
                nc.vector.tensor_tensor(qk_r[:, :, :, 0:32],
                                        qk_nd[:, :, :, 32:64], sn0, OP.mult)
                nc.vector.tensor_tensor(qk_r[:, :, :, 0:32],
                                        tmp[:, :, :, 0:32],
                                        qk_r[:, :, :, 0:32], OP.subtract)
                nc.vector.tensor_tensor(qk_r[:, :, :, 32:64],
                                        qk_nd[:, :, :, 0:32], sn1, OP.mult)
                nc.vector.tensor_tensor(qk_r[:, :, :, 32:64],
                                        tmp[:, :, :, 32:64],
                                        qk_r[:, :, :, 32:64], OP.add)

                return qk_r, vts

            def transpose_tile(t, qk_r, vts):
                """PE transposes: q,k -> [ch, n]; v^T -> V rows."""
                for j in range(4):
                    g = 4 * t + j
                    col = 128 * g
                    ptq = ps.tile([128, 128], F32R, tag="ps",
                                  name=f"ptq{g}")
                    nc.tensor.transpose(ptq[:], qk_r[:, j, 0:2, :], identr[:])
                    nc.vector.tensor_copy(qT[:, col:col + 128], ptq[:])
                    ptk = ps.tile([128, 128], F32R, tag="ps",
                                  name=f"ptk{g}")
                    nc.tensor.transpose(ptk[:], qk_r[:, j, 2:4, :], identr[:])
                    nc.vector.tensor_copy(kT[:, col:col + 128], ptk[:])
                    ptv = ps.tile([128, 128], F32R, tag="ps",
                                  name=f"ptv{g}")
                    nc.tensor.transpose(ptv[:], vts[:, 128 * j:128 * (j + 1)],
                                        identr[:])
                    nc.vector.tensor_copy(
                        vext[:, g, :, 0:64],
                        ptv[:].rearrange("p (h d) -> p h d", h=HPC))

            def attn_pair(b, h):
                """softmax(K Q^T / 8)^T-weighted V for (batch b, local head
                h); writes normalized out^T into attn_outT."""
                col0 = N * b
                hof = D * h
                for qt in range(4):
                    qs = col0 + 512 * qt
                    pav = psav.tile([65, 512], F32, tag="av",
                                    name=f"pav{b}{h}{qt}")
                    for kc in range(16):
                        pS = ps.tile([128, 512], F32, tag="ps",
                                     name=f"pS{b}{h}{qt}_{kc}")
                        nc.tensor.matmul(
                            pS[:],
                            kT[hof:hof + D,
                               col0 + 128 * kc:col0 + 128 * (kc + 1)],
                            qT[hof:hof + D, qs:qs + 512],
                            start=True, stop=True)
                        es = expp.tile([128, 512], F32R, tag="es",
                                       name=f"es{b}{h}{qt}_{kc}")
                        nc.scalar.activation(es[:], pS[:], AF.Exp, scale=0.125)
                        nc.tensor.matmul(pav[:],
                                         vext[:, 16 * b + kc, h, 0:65],
                                         es[:],
                                         start=(kc == 0), stop=(kc == 15))
                    # normalize columns by row 64 (softmax denominator)
                    rec = normp.tile([1, 512], F32, tag="rec",
                                     name=f"rec{b}{h}{qt}")
                    nc.vector.reciprocal(rec[:], pav[64:65, :])
                    recr = normp.tile([1, 512], F32R, tag="recr",
                                      name=f"recr{b}{h}{qt}")
                    nc.scalar.activation(recr[:], rec[:], AF.Copy)
                    pbc = ps.tile([64, 512], F32, tag="ps",
                                  name=f"pbc{b}{h}{qt}")
                    nc.tensor.matmul(pbc[:], onesr[:], recr[:],
                                     start=True, stop=True)
                    bc = normp.tile([64, 512], F32, tag="bc",
                                    name=f"bc{b}{h}{qt}")
                    nc.vector.tensor_copy(bc[:], pbc[:])
                    nc.vector.tensor_tensor(
                        attn_outT[hof:hof + D, qs:qs + 512],
                        pav[0:64, :], bc[:], OP.mult)

            # ---- emit: keep the PE stream dense -------------------------
            # projections run ahead; transposes lag 2 tiles so their DVE
            # dependencies (LN+RoPE) are ready when PE reaches them
            staged = {}
            for t in range(8):
                staged[t] = preamble_tile(t)
                if t - 2 >= 0:
                    transpose_tile(t - 2, *staged.pop(t - 2))
            for t in (6, 7):
                transpose_tile(t, *staged.pop(t))
            for h in range(HPC):
                attn_pair(0, h)
            for h in range(HPC):
                attn_pair(1, h)

            # ---- AllToAll: swap channel axis for row axis -----------------
            ccin = dram.tile([N_CORES, 128, RPC], BF16)
            ccout = dram.tile([N_CORES, 128, RPC], BF16)
            for j in range(N_CORES):
                nc.sync.dma_start(ccin[j], attn_outT[:, RPC * j:RPC * (j + 1)])
            nc.gpsimd.collective_compute(
                "AllToAll", OP.bypass,
                replica_groups=[list(range(N_CORES))],
                ins=[ccin[:].opt()], outs=[ccout[:].opt()])
            ccout_r = ccout[:].rearrange("j p n -> p j n")
            out_r = out_d.ap().rearrange("(t p) o -> p t o", p=128)

            # ---- output projection for this core's 512 rows ---------------
            for nt in range(4):
                gat = freqs.tile([128, 8, 128], BF16, tag="gat",
                                 name=f"gat{nt}")
                nc.sync.dma_start(gat[:],
                                  ccout_r[:, :, 128 * nt:128 * (nt + 1)])
                ob = work.tile([128, C], F32, tag="ob", name=f"ob{nt}")
                for hf in range(2):
                    po = ps.tile([128, 512], F32, tag="ps",
                                 name=f"po{nt}_{hf}")
                    for cc in range(8):
                        nc.tensor.matmul(
                            po[:],
                            gat[:, cc, :],
                            wp_sb[:, cc, 512 * hf:512 * (hf + 1)],
                            start=(cc == 0), stop=(cc == 7))
                    nc.vector.tensor_tensor(
                        ob[:, 512 * hf:512 * (hf + 1)], po[:],
                        bias_sb[:, 512 * hf:512 * (hf + 1)], OP.add)
                nc.sync.dma_start(out_r[:, nt, :], ob[:])
    _split_excess_waits(nc)
    return nc


_NC_CACHE = {}


def _get_nc():
    if "nc" not in _NC_CACHE:
        _NC_CACHE["nc"] = build()
    return _NC_CACHE["nc"]


def _prep_inputs(x, w_qkv, w_proj, b_proj, freqs_cos, freqs_sin):
    x = np.asarray(x, dtype=np.float32)
    w_qkv = np.asarray(w_qkv, dtype=np.float32)
    w_proj = np.asarray(w_proj, dtype=np.float32)
    b_proj = np.asarray(b_proj, dtype=np.float32)
    cos = np.asarray(freqs_cos, dtype=np.float32).reshape(N, D)
    sin = np.asarray(freqs_sin, dtype=np.float32).reshape(N, D)

    bf = ml_dtypes.bfloat16
    xT = np.ascontiguousarray(x.reshape(NTOT, C).T).astype(bf)
    wpT = np.ascontiguousarray(w_proj.T).astype(bf)
    biasb = np.ascontiguousarray(np.broadcast_to(b_proj, (128, C))).astype(np.float32)
    ident = np.eye(128, dtype=np.float32)

    in_maps = []
    for i in range(N_CORES):
        r0 = CPC * i
        wqk = np.concatenate([w_qkv[r0:r0 + CPC],
                              w_qkv[C + r0:C + r0 + CPC]], axis=0)
        wqkT = np.ascontiguousarray(wqk.T).astype(bf)
        wvT = np.ascontiguousarray(w_qkv[2 * C + r0:2 * C + r0 + CPC].T).astype(bf)
        in_maps.append({
            "xT": xT, "wqkT": wqkT, "wvT": wvT, "wpT": wpT,
            "biasb": biasb, "cosd": cos, "sind": sin, "identd": ident,
        })
    return in_maps


def kernel(x, w_qkv, w_proj, b_proj, freqs_cos, freqs_sin):
    in_maps = _prep_inputs(x, w_qkv, w_proj, b_proj, freqs_cos, freqs_sin)
    nc = _get_nc()
    res = run_bass_kernel_spmd(nc, in_maps, core_ids=list(range(N_CORES)))
    full = np.concatenate([res.results[i]["out"] for i in range(N_CORES)],
                          axis=0)
    return full.reshape(B, N, C).astype(np.float32)


# revision 6
# speedup vs baseline: 1.2220x; 1.1237x over previous
"""Trainium2 8-core attention kernel (nn_Attention_19954418057485).

Sharding: heads are split across the 8 cores (2 heads = 128 channels each);
every core processes both batch elements for its heads.  After attention,
an AllToAll over all 8 cores swaps the channel axis for the row axis, so
each core finishes the full output projection for its 512 rows of the
flattened (B*N, C) output.

Per-core pipeline (all matmuls on PE, exp on ACT, elementwise on DVE):
  x^T (bf16)  --PE-->  q,k (rows,ch) + v^T        [QKV projection]
  q,k: LayerNorm (d=64) + RoPE on DVE, rounded to fp32r
  PE transpose -> q^T, k^T [ch, n];  v^T -> V [n, ch] with a ones column
  per (batch, head): S^T = K Q^T (fp32r), exp on ACT (scale=1/8, fp32r),
  AV accumulates V_ext^T @ expS^T giving out^T rows 0..63 and the softmax
  denominator in row 64 (from the ones column); normalize via reciprocal +
  PE outer-product broadcast.
  AllToAll (bf16) -> full-channel rows -> output projection (bf16) + bias.
"""
import sys

if "/opt/trn_rl_repo" not in sys.path:
    sys.path.insert(0, "/opt/trn_rl_repo")

import numpy as np
import ml_dtypes

import concourse.bass as bass
import concourse.tile as tile
from concourse import mybir
from concourse.bass_utils import run_bass_kernel_spmd

N_CORES = 8
B, N, C, H = 2, 2048, 1024, 16
D = 64
HPC = H // N_CORES          # heads per core = 2
CPC = HPC * D               # channels per core = 128
NTOT = B * N                # 4096 flattened rows
RPC = NTOT // N_CORES       # output rows per core = 512
EPS = 1e-6

BF16 = mybir.dt.bfloat16
F32 = mybir.dt.float32
F32R = mybir.dt.float32r
AF = mybir.ActivationFunctionType
OP = mybir.AluOpType
AX = mybir.AxisListType


def _split_excess_waits(nc, max_waits=1):
    """walrus rejects instructions with more than a couple of sem-wait
    commands; split extras onto preceding same-engine NoOps."""
    for fn in nc.m.functions:
        for blk in fn.blocks:
            new_insts = []
            for ins in blk.instructions:
                si = ins.sync_info
                ow = list(si.on_wait) if si is not None and si.on_wait else []
                if len(ow) > max_waits:
                    head = ow[: len(ow) - max_waits]
                    rest = ow[len(ow) - max_waits:]
                    for i in range(0, len(head), max_waits):
                        new_insts.append(mybir.InstNoOp(
                            name=f"{ins.name}_ws{i}",
                            engine=ins.engine,
                            ins=[], outs=[],
                            sync_info=mybir.SyncInfo(
                                on_wait=head[i:i + max_waits], on_update=[]),
                        ))
                    ins.sync_info = mybir.SyncInfo(
                        on_wait=rest, on_update=list(si.on_update or []))
                new_insts.append(ins)
            blk.instructions = new_insts


def build():
    nc = bass.Bass("TRN2", target_bir_lowering=False, debug=False,
                   num_devices=N_CORES)
    xT_d = nc.dram_tensor("xT", (C, NTOT), BF16, kind="ExternalInput")
    wqkT_d = nc.dram_tensor("wqkT", (C, 2 * CPC), BF16, kind="ExternalInput")
    wvT_d = nc.dram_tensor("wvT", (C, CPC), BF16, kind="ExternalInput")
    wpT_d = nc.dram_tensor("wpT", (C, C), BF16, kind="ExternalInput")
    bias_d = nc.dram_tensor("biasb", (128, C), F32, kind="ExternalInput")
    cos_d = nc.dram_tensor("cosd", (N, D), F32, kind="ExternalInput")
    sin_d = nc.dram_tensor("sind", (N, D), F32, kind="ExternalInput")
    ident_d = nc.dram_tensor("identd", (128, 128), F32, kind="ExternalInput")
    out_d = nc.dram_tensor("out", (RPC, C), F32, kind="ExternalOutput")

    with tile.TileContext(nc) as tc:
        with tc.tile_pool(name="consts", bufs=1) as consts, \
             tc.tile_pool(name="xload", bufs=3) as xload, \
             tc.tile_pool(name="vpool", bufs=4) as vpool, \
             tc.tile_pool(name="qkrp", bufs=4) as qkrp, \
             tc.tile_pool(name="freqs", bufs=2) as freqs, \
             tc.tile_pool(name="work", bufs=2) as work, \
             tc.tile_pool(name="small", bufs=3) as small, \
             tc.tile_pool(name="exps", bufs=4) as expp, \
             tc.tile_pool(name="norm", bufs=2) as normp, \
             tc.tile_pool(name="ps", bufs=5, space="PSUM") as ps, \
             tc.tile_pool(name="psav", bufs=2, space="PSUM") as psav, \
             tc.tile_pool(name="dram", bufs=1, space="DRAM") as dram:

            # ---- constants -------------------------------------------------
            wqk_sb = consts.tile([128, 8, 2 * CPC], BF16)
            nc.sync.dma_start(wqk_sb[:],
                              wqkT_d.ap().rearrange("(co p) k -> p co k", p=128))
            wv_sb = consts.tile([128, 8, CPC], BF16)
            nc.sync.dma_start(wv_sb[:],
                              wvT_d.ap().rearrange("(co p) k -> p co k", p=128))
            wp_sb = consts.tile([128, 8, C], BF16)
            bias_sb = consts.tile([128, C], F32)
            nc.sync.dma_start(bias_sb[:], bias_d.ap())
            cos_r = cos_d.ap().rearrange("(c p) d -> p c d", p=128)
            sin_r = sin_d.ap().rearrange("(c p) d -> p c d", p=128)
            ident_f = consts.tile([128, 128], F32)
            nc.sync.dma_start(ident_f[:], ident_d.ap())
            identr = consts.tile([128, 128], BF16)
            nc.scalar.activation(identr[:], ident_f[:], AF.Copy)
            onesr = consts.tile([1, 64], BF16)
            nc.scalar.activation(onesr[:], ident_f[0:1, 0:64], AF.Identity,
                                 scale=0.0, bias=1.0)

            # ---- persistent tensors ---------------------------------------
            qT = consts.tile([128, NTOT], BF16)       # [ch(2 heads), b*N+n]
            kT = consts.tile([128, NTOT], BF16)
            # V with ones column: [n%128, chunk, head, 64 d + 1 one + pad]
            vext = consts.tile([128, 32, HPC, 66], BF16)
            nc.scalar.activation(
                vext[:, :, :, 64:65],
                bias_sb[:, 0:64].rearrange("p (a b c) -> p a b c", a=32, b=2),
                AF.Identity, scale=0.0, bias=1.0)
            attn_outT = consts.tile([128, NTOT], BF16)

            xT_r = xT_d.ap().rearrange("(co p) n -> p co n", p=128)

            def preamble_tile(t):
                """QKV proj + LN + RoPE + transposes for n-rows
                [t*512, (t+1)*512)."""
                xt = xload.tile([128, 8, 512], BF16, tag="xt", name=f"xt{t}")
                nc.sync.dma_start(xt[:], xT_r[:, :, 512 * t:512 * (t + 1)])

                qk_nd = work.tile([128, 4, 4, D], F32, tag="qknd",
                                  name=f"qknd{t}")
                for ns in range(4):
                    pj = ps.tile([128, 2 * CPC], F32, tag="ps",
                                 name=f"pj{t}_{ns}")
                    for cc in range(8):
                        nc.tensor.matmul(pj[:],
                                         xt[:, cc, 128 * ns:128 * (ns + 1)],
                                         wqk_sb[:, cc, :],
                                         start=(cc == 0), stop=(cc == 7))
                    nc.vector.tensor_copy(
                        qk_nd[:, ns],
                        pj[:].rearrange("p (s d) -> p s d", s=4))

                pv = ps.tile([128, 512], F32, tag="ps", name=f"pv{t}")
                for cc in range(8):
                    nc.tensor.matmul(pv[:], wv_sb[:, cc, :], xt[:, cc, :],
                                     start=(cc == 0), stop=(cc == 7))
                vts = vpool.tile([128, 512], BF16, tag="vts", name=f"vts{t}")
                nc.vector.tensor_copy(vts[:], pv[:])

                # LayerNorm stats over d=64 for each (row, slot)
                s1 = small.tile([128, 4, 4], F32, tag="s1", name=f"s1_{t}")
                nc.vector.reduce_sum(s1[:], qk_nd[:], axis=AX.X)
                sq = work.tile([128, 4, 4, D], F32, tag="tmp", name=f"sq{t}")
                nc.vector.tensor_tensor(sq[:], qk_nd[:], qk_nd[:], OP.mult)
                s2 = small.tile([128, 4, 4], F32, tag="s2", name=f"s2_{t}")
                nc.vector.reduce_sum(s2[:], sq[:], axis=AX.X)
                mu = small.tile([128, 4, 4], F32, tag="mu", name=f"mu{t}")
                nc.vector.tensor_scalar_mul(mu[:], s1[:], 1.0 / D)
                var = small.tile([128, 4, 4], F32, tag="var", name=f"var{t}")
                nc.vector.tensor_scalar_mul(var[:], s2[:], 1.0 / D)
                mm = small.tile([128, 4, 4], F32, tag="mm", name=f"mm{t}")
                nc.vector.tensor_tensor(mm[:], mu[:], mu[:], OP.mult)
                nc.vector.tensor_tensor(var[:], var[:], mm[:], OP.subtract)
                nc.vector.tensor_scalar_add(var[:], var[:], EPS)
                # rsqrt(var+eps) = exp(-0.5*ln(var+eps)): stays in the
                # ln/exp ACT table set (no thrash against attention's Exp)
                lnv = small.tile([128, 4, 4], F32, tag="lnv", name=f"lnv{t}")
                nc.scalar.activation(lnv[:], var[:], AF.Ln)
                a_ = small.tile([128, 4, 4], F32, tag="a", name=f"a{t}")
                nc.scalar.activation(a_[:], lnv[:], AF.Exp, scale=-0.5)
                nma = small.tile([128, 4, 4], F32, tag="nma", name=f"nma{t}")
                nc.vector.tensor_tensor(nma[:], mu[:], a_[:], OP.mult)
                # qn = q*a - mu*a
                nc.vector.tensor_tensor(
                    qk_nd[:], qk_nd[:],
                    a_[:, :, :, None].to_broadcast((128, 4, 4, D)), OP.mult)
                nc.vector.tensor_tensor(
                    qk_nd[:], qk_nd[:],
                    nma[:, :, :, None].to_broadcast((128, 4, 4, D)),
                    OP.subtract)

                # RoPE: out = qn*cos + rot_half(qn)*sin  (fp32r output)
                tb = t % 4
                cos_t = freqs.tile([128, 4, D], F32, tag="cos", name=f"cos{t}")
                nc.sync.dma_start(cos_t[:], cos_r[:, 4 * tb:4 * (tb + 1), :])
                sin_t = freqs.tile([128, 4, D], F32, tag="sin", name=f"sin{t}")
                nc.sync.dma_start(sin_t[:], sin_r[:, 4 * tb:4 * (tb + 1), :])
                cs = cos_t[:, :, None, :].to_broadcast((128, 4, 4, D))
                sn0 = sin_t[:, :, None, 0:32].to_broadcast((128, 4, 4, 32))
                sn1 = sin_t[:, :, None, 32:64].to_broadcast((128, 4, 4, 32))
                tmp = work.tile([128, 4, 4, D], F32, tag="tmp", name=f"tmp{t}")
                nc.vector.tensor_tensor(tmp[:], qk_nd[:], cs, OP.mult)
                qk_r = qkrp.tile([128, 4, 4, D], BF16, tag="qkr",
                                 name=f"qkr{t}")
                nc.vector.tensor_tensor(qk_r[:, :, :, 0:32],
                                        qk_nd[:, :, :, 32:64], sn0, OP.mult)
                nc.vector.tensor_tensor(qk_r[:, :, :, 0:32],
                                        tmp[:, :, :, 0:32],
                                        qk_r[:, :, :, 0:32], OP.subtract)
                nc.vector.tensor_tensor(qk_r[:, :, :, 32:64],
                                        qk_nd[:, :, :, 0:32], sn1, OP.mult)
                nc.vector.tensor_tensor(qk_r[:, :, :, 32:64],
                                        tmp[:, :, :, 32:64],
                                        qk_r[:, :, :, 32:64], OP.add)

                return qk_r, vts

            def transpose_tile(t, qk_r, vts):
                """PE transposes: q,k -> [ch, n]; v^T -> V rows."""
                for j in range(4):
                    g = 4 * t + j
                    col = 128 * g
                    ptq = ps.tile([128, 128], BF16, tag="ps",
                                  name=f"ptq{g}")
                    nc.tensor.transpose(ptq[:], qk_r[:, j, 0:2, :], identr[:])
                    nc.vector.tensor_copy(qT[:, col:col + 128], ptq[:])
                    ptk = ps.tile([128, 128], BF16, tag="ps",
                                  name=f"ptk{g}")
                    nc.tensor.transpose(ptk[:], qk_r[:, j, 2:4, :], identr[:])
                    nc.vector.tensor_copy(kT[:, col:col + 128], ptk[:])
                    ptv = ps.tile([128, 128], BF16, tag="ps",
                                  name=f"ptv{g}")
                    nc.tensor.transpose(ptv[:], vts[:, 128 * j:128 * (j + 1)],
                                        identr[:])
                    nc.vector.tensor_copy(
                        vext[:, g, :, 0:64],
                        ptv[:].rearrange("p (h d) -> p h d", h=HPC))

            def attn_pair(b, h):
                """softmax(K Q^T / 8)^T-weighted V for (batch b, local head
                h); writes normalized out^T into attn_outT."""
                col0 = N * b
                hof = D * h
                for qt in range(4):
                    qs = col0 + 512 * qt
                    pav = psav.tile([65, 512], F32, tag="av",
                                    name=f"pav{b}{h}{qt}")
                    for kc in range(16):
                        pS = ps.tile([128, 512], F32, tag="ps",
                                     name=f"pS{b}{h}{qt}_{kc}")
                        nc.tensor.matmul(
                            pS[:],
                            kT[hof:hof + D,
                               col0 + 128 * kc:col0 + 128 * (kc + 1)],
                            qT[hof:hof + D, qs:qs + 512],
                            start=True, stop=True)
                        es = expp.tile([128, 512], BF16, tag="es",
                                       name=f"es{b}{h}{qt}_{kc}")
                        nc.scalar.activation(es[:], pS[:], AF.Exp, scale=0.125)
                        nc.tensor.matmul(pav[:],
                                         vext[:, 16 * b + kc, h, 0:65],
                                         es[:],
                                         start=(kc == 0), stop=(kc == 15))
                    # normalize columns by row 64 (softmax denominator)
                    rec = normp.tile([1, 512], F32, tag="rec",
                                     name=f"rec{b}{h}{qt}")
                    nc.vector.reciprocal(rec[:], pav[64:65, :])
                    recr = normp.tile([1, 512], BF16, tag="recr",
                                      name=f"recr{b}{h}{qt}")
                    nc.vector.tensor_copy(recr[:], rec[:])
                    pbc = ps.tile([64, 512], F32, tag="ps",
                                  name=f"pbc{b}{h}{qt}")
                    nc.tensor.matmul(pbc[:], onesr[:], recr[:],
                                     start=True, stop=True)
                    bc = normp.tile([64, 512], F32, tag="bc",
                                    name=f"bc{b}{h}{qt}")
                    nc.vector.tensor_copy(bc[:], pbc[:])
                    nc.vector.tensor_tensor(
                        attn_outT[hof:hof + D, qs:qs + 512],
                        pav[0:64, :], bc[:], OP.mult)

            # ---- emit: keep the PE stream dense -------------------------
            # projections run ahead; transposes lag 2 tiles so their DVE
            # dependencies (LN+RoPE) are ready when PE reaches them
            staged = {}
            for t in range(8):
                staged[t] = preamble_tile(t)
                if t - 2 >= 0:
                    transpose_tile(t - 2, *staged.pop(t - 2))
            for t in (6, 7):
                transpose_tile(t, *staged.pop(t))
            nc.sync.dma_start(wp_sb[:],
                              wpT_d.ap().rearrange("(co p) k -> p co k", p=128))
            for h in range(HPC):
                attn_pair(0, h)
            for h in range(HPC):
                attn_pair(1, h)

            # ---- AllToAll: swap channel axis for row axis -----------------
            ccin = dram.tile([N_CORES, 128, RPC], BF16)
            ccout = dram.tile([N_CORES, 128, RPC], BF16)
            for j in range(N_CORES):
                nc.sync.dma_start(ccin[j], attn_outT[:, RPC * j:RPC * (j + 1)])
            nc.gpsimd.collective_compute(
                "AllToAll", OP.bypass,
                replica_groups=[list(range(N_CORES))],
                ins=[ccin[:].opt()], outs=[ccout[:].opt()])
            ccout_r = ccout[:].rearrange("j p n -> p j n")
            out_r = out_d.ap().rearrange("(t p) o -> p t o", p=128)

            # ---- output projection for this core's 512 rows ---------------
            for nt in range(4):
                gat = freqs.tile([128, 8, 128], BF16, tag="gat",
                                 name=f"gat{nt}")
                nc.sync.dma_start(gat[:],
                                  ccout_r[:, :, 128 * nt:128 * (nt + 1)])
                ob = work.tile([128, C], F32, tag="ob", name=f"ob{nt}")
                for hf in range(2):
                    po = ps.tile([128, 512], F32, tag="ps",
                                 name=f"po{nt}_{hf}")
                    for cc in range(8):
                        nc.tensor.matmul(
                            po[:],
                            gat[:, cc, :],
                            wp_sb[:, cc, 512 * hf:512 * (hf + 1)],
                            start=(cc == 0), stop=(cc == 7))
                    nc.vector.tensor_tensor(
                        ob[:, 512 * hf:512 * (hf + 1)], po[:],
                        bias_sb[:, 512 * hf:512 * (hf + 1)], OP.add)
                nc.sync.dma_start(out_r[:, nt, :], ob[:])
    _split_excess_waits(nc)
    return nc


_NC_CACHE = {}


def _get_nc():
    if "nc" not in _NC_CACHE:
        _NC_CACHE["nc"] = build()
    return _NC_CACHE["nc"]


def _prep_inputs(x, w_qkv, w_proj, b_proj, freqs_cos, freqs_sin):
    x = np.asarray(x, dtype=np.float32)
    w_qkv = np.asarray(w_qkv, dtype=np.float32)
    w_proj = np.asarray(w_proj, dtype=np.float32)
    b_proj = np.asarray(b_proj, dtype=np.float32)
    cos = np.asarray(freqs_cos, dtype=np.float32).reshape(N, D)
    sin = np.asarray(freqs_sin, dtype=np.float32).reshape(N, D)

    bf = ml_dtypes.bfloat16
    xT = np.ascontiguousarray(x.reshape(NTOT, C).T).astype(bf)
    wpT = np.ascontiguousarray(w_proj.T).astype(bf)
    biasb = np.ascontiguousarray(np.broadcast_to(b_proj, (128, C))).astype(np.float32)
    ident = np.eye(128, dtype=np.float32)

    in_maps = []
    for i in range(N_CORES):
        r0 = CPC * i
        wqk = np.concatenate([w_qkv[r0:r0 + CPC],
                              w_qkv[C + r0:C + r0 + CPC]], axis=0)
        wqkT = np.ascontiguousarray(wqk.T).astype(bf)
        wvT = np.ascontiguousarray(w_qkv[2 * C + r0:2 * C + r0 + CPC].T).astype(bf)
        in_maps.append({
            "xT": xT, "wqkT": wqkT, "wvT": wvT, "wpT": wpT,
            "biasb": biasb, "cosd": cos, "sind": sin, "identd": ident,
        })
    return in_maps


def kernel(x, w_qkv, w_proj, b_proj, freqs_cos, freqs_sin):
    in_maps = _prep_inputs(x, w_qkv, w_proj, b_proj, freqs_cos, freqs_sin)
    nc = _get_nc()
    res = run_bass_kernel_spmd(nc, in_maps, core_ids=list(range(N_CORES)))
    full = np.concatenate([res.results[i]["out"] for i in range(N_CORES)],
                          axis=0)
    return full.reshape(B, N, C).astype(np.float32)


# revision 7
# speedup vs baseline: 1.9355x; 1.5839x over previous
"""Trainium2 8-core attention kernel (nn_Attention_19954418057485).

Sharding: heads are split across the 8 cores (2 heads = 128 channels each);
every core processes both batch elements for its heads.  After attention,
an AllToAll over all 8 cores swaps the channel axis for the row axis, so
each core finishes the full output projection for its 512 rows of the
flattened (B*N, C) output.

Per-core pipeline (matmuls on PE in bf16, exp on ACT, elementwise on DVE):
  x^T (bf16)  --PE-->  q,k (rows,ch) + v^T        [QKV projection]
  q,k: LayerNorm (d=64) + RoPE (bf16 DVE ops), then PE transposes to
  q^T,k^T [ch, n]; v^T -> V [n, ch] with a ones column appended.
  per (batch, head): S^T = K Q^T, exp(S/8) on ACT (no max-subtraction
  needed: layernormed q,k bound |scores| <= 8), AV accumulates
  V_ext^T @ expS^T giving out^T rows 0..63 plus the softmax denominator
  in row 64 (from the ones column).  Normalization: denominator row ->
  PE outer-product broadcast -> 1/x via ACT exp(-ln(x)) (same ACT table
  set as Exp; DVE reciprocal is 3.4us/op) -> one DVE multiply.
  AllToAll (bf16) -> full-channel rows -> output projection (bf16)+bias.
"""
import sys

if "/opt/trn_rl_repo" not in sys.path:
    sys.path.insert(0, "/opt/trn_rl_repo")

import numpy as np
import ml_dtypes

import concourse.bass as bass
import concourse.tile as tile
from concourse import mybir
from concourse.bass_utils import run_bass_kernel_spmd

N_CORES = 8
B, N, C, H = 2, 2048, 1024, 16
D = 64
HPC = H // N_CORES          # heads per core = 2
CPC = HPC * D               # channels per core = 128
NTOT = B * N                # 4096 flattened rows
RPC = NTOT // N_CORES       # output rows per core = 512
EPS = 1e-6

BF16 = mybir.dt.bfloat16
F32 = mybir.dt.float32
AF = mybir.ActivationFunctionType
OP = mybir.AluOpType
AX = mybir.AxisListType


def _split_excess_waits(nc, max_waits=1):
    """walrus rejects instructions with more than a couple of sem-wait
    commands; split extras onto preceding same-engine NoOps."""
    for fn in nc.m.functions:
        for blk in fn.blocks:
            new_insts = []
            for ins in blk.instructions:
                si = ins.sync_info
                ow = list(si.on_wait) if si is not None and si.on_wait else []
                if len(ow) > max_waits:
                    head = ow[: len(ow) - max_waits]
                    rest = ow[len(ow) - max_waits:]
                    for i in range(0, len(head), max_waits):
                        new_insts.append(mybir.InstNoOp(
                            name=f"{ins.name}_ws{i}",
                            engine=ins.engine,
                            ins=[], outs=[],
                            sync_info=mybir.SyncInfo(
                                on_wait=head[i:i + max_waits], on_update=[]),
                        ))
                    ins.sync_info = mybir.SyncInfo(
                        on_wait=rest, on_update=list(si.on_update or []))
                new_insts.append(ins)
            blk.instructions = new_insts


def build():
    nc = bass.Bass("TRN2", target_bir_lowering=False, debug=False,
                   num_devices=N_CORES)
    xT_d = nc.dram_tensor("xT", (C, NTOT), BF16, kind="ExternalInput")
    wqkT_d = nc.dram_tensor("wqkT", (C, 2 * CPC), BF16, kind="ExternalInput")
    wvT_d = nc.dram_tensor("wvT", (C, CPC), BF16, kind="ExternalInput")
    wpT_d = nc.dram_tensor("wpT", (C, C), BF16, kind="ExternalInput")
    bias_d = nc.dram_tensor("biasb", (128, C), F32, kind="ExternalInput")
    cos_d = nc.dram_tensor("cosd", (N, D), BF16, kind="ExternalInput")
    sin_d = nc.dram_tensor("sind", (N, D), BF16, kind="ExternalInput")
    ident_d = nc.dram_tensor("identd", (128, 128), F32, kind="ExternalInput")
    out_d = nc.dram_tensor("out", (RPC, C), F32, kind="ExternalOutput")

    with tile.TileContext(nc) as tc:
        with tc.tile_pool(name="consts", bufs=1) as consts, \
             tc.tile_pool(name="xload", bufs=3) as xload, \
             tc.tile_pool(name="vpool", bufs=4) as vpool, \
             tc.tile_pool(name="qkrp", bufs=4) as qkrp, \
             tc.tile_pool(name="freqs", bufs=2) as freqs, \
             tc.tile_pool(name="work", bufs=2) as work, \
             tc.tile_pool(name="small", bufs=3) as small, \
             tc.tile_pool(name="exps", bufs=4) as expp, \
             tc.tile_pool(name="norm", bufs=2) as normp, \
             tc.tile_pool(name="ps", bufs=2, space="PSUM") as ps, \
             tc.tile_pool(name="psS", bufs=2, space="PSUM") as psSp, \
             tc.tile_pool(name="psav", bufs=2, space="PSUM") as psav, \
             tc.tile_pool(name="dram", bufs=1, space="DRAM") as dram:

            # ---- constants -------------------------------------------------
            wqk_sb = consts.tile([128, 8, 2 * CPC], BF16)
            nc.sync.dma_start(wqk_sb[:],
                              wqkT_d.ap().rearrange("(co p) k -> p co k", p=128))
            wv_sb = consts.tile([128, 8, CPC], BF16)
            nc.sync.dma_start(wv_sb[:],
                              wvT_d.ap().rearrange("(co p) k -> p co k", p=128))
            wp_sb = consts.tile([128, 8, C], BF16)      # DMA deferred
            bias_sb = consts.tile([128, C], F32)
            nc.sync.dma_start(bias_sb[:], bias_d.ap())
            cos_r = cos_d.ap().rearrange("(c p) d -> p c d", p=128)
            sin_r = sin_d.ap().rearrange("(c p) d -> p c d", p=128)
            ident_f = consts.tile([128, 128], F32)
            nc.sync.dma_start(ident_f[:], ident_d.ap())
            identr = consts.tile([128, 128], BF16)
            nc.scalar.activation(identr[:], ident_f[:], AF.Copy)
            onesr = consts.tile([1, 64], BF16)
            nc.scalar.activation(onesr[:], ident_f[0:1, 0:64], AF.Identity,
                                 scale=0.0, bias=1.0)

            # ---- persistent tensors ---------------------------------------
            qkT = consts.tile([128, 2, NTOT], BF16)   # [ch, {q,k}, b*N+n]
            # V with ones column: [n%128, chunk, head, 64 d + 1 one + pad]
            vext = consts.tile([128, 32, HPC, 66], BF16)
            nc.scalar.activation(
                vext[:, :, :, 64:65],
                bias_sb[:, 0:64].rearrange("p (a b c) -> p a b c", a=32, b=2),
                AF.Identity, scale=0.0, bias=1.0)
            attn_outT = consts.tile([128, NTOT], BF16)

            xT_r = xT_d.ap().rearrange("(co p) n -> p co n", p=128)

            def preamble_tile(t):
                """QKV proj + LN + RoPE for n-rows [t*512, (t+1)*512)."""
                xt = xload.tile([128, 8, 512], BF16, tag="xt", name=f"xt{t}")
                nc.sync.dma_start(xt[:], xT_r[:, :, 512 * t:512 * (t + 1)])

                qk_nd = work.tile([128, 4, 4, D], BF16, tag="qknd",
                                  name=f"qknd{t}")
                for ns in range(4):
                    pj = ps.tile([128, 2 * CPC], F32, tag="ps",
                                 name=f"pj{t}_{ns}")
                    for cc in range(8):
                        nc.tensor.matmul(pj[:],
                                         xt[:, cc, 128 * ns:128 * (ns + 1)],
                                         wqk_sb[:, cc, :],
                                         start=(cc == 0), stop=(cc == 7))
                    nc.scalar.activation(
                        qk_nd[:, ns],
                        pj[:].rearrange("p (s d) -> p s d", s=4), AF.Copy)

                pv = ps.tile([128, 512], F32, tag="ps", name=f"pv{t}")
                for cc in range(8):
                    nc.tensor.matmul(pv[:], wv_sb[:, cc, :], xt[:, cc, :],
                                     start=(cc == 0), stop=(cc == 7))
                vts = vpool.tile([128, 512], BF16, tag="vts", name=f"vts{t}")
                nc.scalar.activation(vts[:], pv[:], AF.Copy)

                # LayerNorm stats over d=64 for each (row, slot)
                s1 = small.tile([128, 4, 4], F32, tag="s1", name=f"s1_{t}")
                nc.vector.reduce_sum(s1[:], qk_nd[:], axis=AX.X)
                sq = work.tile([128, 4, 4, D], BF16, tag="tmp", name=f"sq{t}")
                nc.vector.tensor_tensor(sq[:], qk_nd[:], qk_nd[:], OP.mult)
                s2 = small.tile([128, 4, 4], F32, tag="s2", name=f"s2_{t}")
                nc.vector.reduce_sum(s2[:], sq[:], axis=AX.X)
                mu = small.tile([128, 4, 4], F32, tag="mu", name=f"mu{t}")
                nc.vector.tensor_scalar_mul(mu[:], s1[:], 1.0 / D)
                var = small.tile([128, 4, 4], F32, tag="var", name=f"var{t}")
                nc.vector.tensor_scalar_mul(var[:], s2[:], 1.0 / D)
                mm = small.tile([128, 4, 4], F32, tag="mm", name=f"mm{t}")
                nc.vector.tensor_tensor(mm[:], mu[:], mu[:], OP.mult)
                nc.vector.tensor_tensor(var[:], var[:], mm[:], OP.subtract)
                nc.vector.tensor_scalar_add(var[:], var[:], EPS)
                # rsqrt(var+eps) = exp(-0.5*ln(var+eps)): stays in the
                # ln/exp ACT table set (no thrash against attention's Exp)
                lnv = small.tile([128, 4, 4], F32, tag="lnv", name=f"lnv{t}")
                nc.scalar.activation(lnv[:], var[:], AF.Ln)
                a_ = small.tile([128, 4, 4], BF16, tag="a", name=f"a{t}")
                nc.scalar.activation(a_[:], lnv[:], AF.Exp, scale=-0.5)
                nma = small.tile([128, 4, 4], BF16, tag="nma", name=f"nma{t}")
                nc.vector.tensor_tensor(nma[:], mu[:], a_[:], OP.mult)
                # qn = q*a - mu*a
                nc.vector.tensor_tensor(
                    qk_nd[:], qk_nd[:],
                    a_[:, :, :, None].to_broadcast((128, 4, 4, D)), OP.mult)
                nc.vector.tensor_tensor(
                    qk_nd[:], qk_nd[:],
                    nma[:, :, :, None].to_broadcast((128, 4, 4, D)),
                    OP.subtract)

                # RoPE: out = qn*cos + rot_half(qn)*sin
                tb = t % 4
                cos_t = freqs.tile([128, 4, D], BF16, tag="cos", name=f"cos{t}")
                nc.sync.dma_start(cos_t[:], cos_r[:, 4 * tb:4 * (tb + 1), :])
                sin_t = freqs.tile([128, 4, D], BF16, tag="sin", name=f"sin{t}")
                nc.sync.dma_start(sin_t[:], sin_r[:, 4 * tb:4 * (tb + 1), :])
                cs = cos_t[:, :, None, :].to_broadcast((128, 4, 4, D))
                sn0 = sin_t[:, :, None, 0:32].to_broadcast((128, 4, 4, 32))
                sn1 = sin_t[:, :, None, 32:64].to_broadcast((128, 4, 4, 32))
                tmp = work.tile([128, 4, 4, D], BF16, tag="tmp", name=f"tmp{t}")
                nc.vector.tensor_tensor(tmp[:], qk_nd[:], cs, OP.mult)
                qk_r = qkrp.tile([128, 4, 4, D], BF16, tag="qkr",
                                 name=f"qkr{t}")
                nc.vector.tensor_tensor(qk_r[:, :, :, 0:32],
                                        qk_nd[:, :, :, 32:64], sn0, OP.mult)
                nc.vector.tensor_tensor(qk_r[:, :, :, 0:32],
                                        tmp[:, :, :, 0:32],
                                        qk_r[:, :, :, 0:32], OP.subtract)
                nc.vector.tensor_tensor(qk_r[:, :, :, 32:64],
                                        qk_nd[:, :, :, 0:32], sn1, OP.mult)
                nc.vector.tensor_tensor(qk_r[:, :, :, 32:64],
                                        tmp[:, :, :, 32:64],
                                        qk_r[:, :, :, 32:64], OP.add)
                return qk_r, vts

            def transpose_tile(t, qk_r, vts):
                """PE transposes: q,k -> [ch, {q,k}, n]; v^T -> V rows."""
                for j in range(4):
                    g = 4 * t + j
                    col = 128 * g
                    ptqk = ps.tile([128, 2, 128], BF16, tag="ps",
                                   name=f"ptqk{g}")
                    nc.tensor.transpose(ptqk[:, 0, :], qk_r[:, j, 0:2, :],
                                        identr[:])
                    nc.tensor.transpose(ptqk[:, 1, :], qk_r[:, j, 2:4, :],
                                        identr[:])
                    nc.vector.tensor_copy(qkT[:, :, col:col + 128], ptqk[:])
                    ptv = ps.tile([128, 128], BF16, tag="ps",
                                  name=f"ptv{g}")
                    nc.tensor.transpose(ptv[:], vts[:, 128 * j:128 * (j + 1)],
                                        identr[:])
                    nc.scalar.activation(
                        vext[:, g, :, 0:64],
                        ptv[:].rearrange("p (h d) -> p h d", h=HPC), AF.Copy)

            def attn_pair(b, h):
                """softmax(K Q^T / 8)^T-weighted V for (batch b, local head
                h); writes normalized out^T into attn_outT."""
                col0 = N * b
                hof = D * h
                for qt in range(4):
                    qs = col0 + 512 * qt
                    pav = psav.tile([65, 512], F32, tag="av",
                                    name=f"pav{b}{h}{qt}")
                    for g in range(8):
                        psS = psSp.tile([128, 2, 512], F32, tag="pss",
                                        name=f"pS{b}{h}{qt}_{g}")
                        for j in range(2):
                            kc = 2 * g + j
                            nc.tensor.matmul(
                                psS[:, j, :],
                                qkT[hof:hof + D, 1,
                                    col0 + 128 * kc:col0 + 128 * (kc + 1)],
                                qkT[hof:hof + D, 0, qs:qs + 512],
                                start=True, stop=True)
                        es = expp.tile([128, 2, 512], BF16, tag="es",
                                       name=f"es{b}{h}{qt}_{g}")
                        nc.scalar.activation(es[:], psS[:], AF.Exp,
                                             scale=0.125)
                        for j in range(2):
                            nc.tensor.matmul(
                                pav[:],
                                vext[:, 16 * b + 2 * g + j, h, 0:65],
                                es[:, j, :],
                                start=(g == 0 and j == 0),
                                stop=(g == 7 and j == 1))
                    # normalize columns by row 64 (softmax denominator):
                    # broadcast den via PE outer product, invert on ACT
                    den = normp.tile([1, 512], BF16, tag="den",
                                     name=f"den{b}{h}{qt}")
                    nc.vector.tensor_copy(den[:], pav[64:65, :])
                    pbc = ps.tile([64, 512], F32, tag="ps",
                                  name=f"pbc{b}{h}{qt}")
                    nc.tensor.matmul(pbc[:], onesr[:], den[:],
                                     start=True, stop=True)
                    lnd = normp.tile([64, 512], F32, tag="lnd",
                                     name=f"lnd{b}{h}{qt}")
                    nc.scalar.activation(lnd[:], pbc[:], AF.Ln)
                    bcr = normp.tile([64, 512], F32, tag="bcr",
                                     name=f"bcr{b}{h}{qt}")
                    nc.scalar.activation(bcr[:], lnd[:], AF.Exp, scale=-1.0)
                    nc.vector.tensor_tensor(
                        attn_outT[hof:hof + D, qs:qs + 512],
                        pav[0:64, :], bcr[:], OP.mult)

            # ---- emit: keep the PE stream dense -------------------------
            # projections run ahead; transposes lag 2 tiles so their DVE
            # dependencies (LN+RoPE) are ready when PE reaches them
            staged = {}
            for t in range(8):
                staged[t] = preamble_tile(t)
                if t - 2 >= 0:
                    transpose_tile(t - 2, *staged.pop(t - 2))
            for t in (6, 7):
                transpose_tile(t, *staged.pop(t))
            nc.sync.dma_start(wp_sb[:],
                              wpT_d.ap().rearrange("(co p) k -> p co k", p=128))
            for h in range(HPC):
                attn_pair(0, h)
            for h in range(HPC):
                attn_pair(1, h)

            # ---- AllToAll: swap channel axis for row axis -----------------
            ccin = dram.tile([N_CORES, 128, RPC], BF16)
            ccout = dram.tile([N_CORES, 128, RPC], BF16)
            for j in range(N_CORES):
                nc.sync.dma_start(ccin[j], attn_outT[:, RPC * j:RPC * (j + 1)])
            nc.gpsimd.collective_compute(
                "AllToAll", OP.bypass,
                replica_groups=[list(range(N_CORES))],
                ins=[ccin[:].opt()], outs=[ccout[:].opt()])
            ccout_r = ccout[:].rearrange("j p n -> p j n")
            out_r = out_d.ap().rearrange("(t p) o -> p t o", p=128)

            # ---- output projection for this core's 512 rows ---------------
            for nt in range(4):
                gat = freqs.tile([128, 8, 128], BF16, tag="gat",
                                 name=f"gat{nt}")
                nc.sync.dma_start(gat[:],
                                  ccout_r[:, :, 128 * nt:128 * (nt + 1)])
                ob = work.tile([128, C], F32, tag="ob", name=f"ob{nt}")
                for hf in range(2):
                    po = ps.tile([128, 512], F32, tag="ps",
                                 name=f"po{nt}_{hf}")
                    for cc in range(8):
                        nc.tensor.matmul(
                            po[:],
                            gat[:, cc, :],
                            wp_sb[:, cc, 512 * hf:512 * (hf + 1)],
                            start=(cc == 0), stop=(cc == 7))
                    nc.vector.tensor_tensor(
                        ob[:, 512 * hf:512 * (hf + 1)], po[:],
                        bias_sb[:, 512 * hf:512 * (hf + 1)], OP.add)
                nc.sync.dma_start(out_r[:, nt, :], ob[:])
    _split_excess_waits(nc)
    return nc


_NC_CACHE = {}


def _get_nc():
    if "nc" not in _NC_CACHE:
        _NC_CACHE["nc"] = build()
    return _NC_CACHE["nc"]


def _prep_inputs(x, w_qkv, w_proj, b_proj, freqs_cos, freqs_sin):
    x = np.asarray(x, dtype=np.float32)
    w_qkv = np.asarray(w_qkv, dtype=np.float32)
    w_proj = np.asarray(w_proj, dtype=np.float32)
    b_proj = np.asarray(b_proj, dtype=np.float32)
    bf = ml_dtypes.bfloat16
    cos = np.asarray(freqs_cos, dtype=np.float32).reshape(N, D).astype(bf)
    sin = np.asarray(freqs_sin, dtype=np.float32).reshape(N, D).astype(bf)

    xT = np.ascontiguousarray(x.reshape(NTOT, C).T).astype(bf)
    wpT = np.ascontiguousarray(w_proj.T).astype(bf)
    biasb = np.ascontiguousarray(
        np.broadcast_to(b_proj, (128, C))).astype(np.float32)
    ident = np.eye(128, dtype=np.float32)

    in_maps = []
    for i in range(N_CORES):
        r0 = CPC * i
        wqk = np.concatenate([w_qkv[r0:r0 + CPC],
                              w_qkv[C + r0:C + r0 + CPC]], axis=0)
        wqkT = np.ascontiguousarray(wqk.T).astype(bf)
        wvT = np.ascontiguousarray(
            w_qkv[2 * C + r0:2 * C + r0 + CPC].T).astype(bf)
        in_maps.append({
            "xT": xT, "wqkT": wqkT, "wvT": wvT, "wpT": wpT,
            "biasb": biasb, "cosd": cos, "sind": sin, "identd": ident,
        })
    return in_maps


def kernel(x, w_qkv, w_proj, b_proj, freqs_cos, freqs_sin):
    in_maps = _prep_inputs(x, w_qkv, w_proj, b_proj, freqs_cos, freqs_sin)
    nc = _get_nc()
    res = run_bass_kernel_spmd(nc, in_maps, core_ids=list(range(N_CORES)))
    full = np.concatenate([res.results[i]["out"] for i in range(N_CORES)],
                          axis=0)
    return full.reshape(B, N, C).astype(np.float32)
